# revision 5
# baseline (speedup 1.0000x reference)
"""CTC loss Trainium2 kernel (Bass/Tile), pure data-parallel over 8 NeuronCores.

Contract: kernel(y_true [2048,32] i32, y_pred [2048,256,128] f32) -> loss [2048] f32.

Algorithm per core (256 examples = 2 blocks of 128, examples on partitions):
  Stream y in 32 tiles [128=(8 ex, 16 t_hi), 2048=(16 t_lo, 128 c)]:
    E = exp(y)                     ACT -> bf16
    Z[b,t] = sum_c E               DVE halve-add (bf16 2x) + reduce
    logZ + per-tile accumulation   ACT Ln with accum_out
    gather y at the 33 classes     GPSIMD ap_gather (f32, per-16-partition indices)
    fold gather -> q[128 ex, 256 t, 33 j]   16 SBUF->SBUF DMAs per tile
  Per block of 128 examples:
    p'' = exp(q + PBIAS)           ACT -> bf16  (PBIAS recenters alpha drift)
    CTC forward DP as 65 tensor_tensor_scan ops along time:
      alpha_t[s] = (u_t + alpha_{t-1}[s]) * p''_t[s]
      u = alpha[s-1] (+ mask_i * alpha[s-2] for odd s>=3, via scalar_tensor_tensor)
    loss = sum_t log Z_t - T*|PBIAS| - log(alpha_T[63] + alpha_T[64])
"""
import sys

sys.path.insert(0, "/opt/trn_rl_repo")
import numpy as np

import concourse.bacc as bacc
import concourse.mybir as mybir
import concourse.tile as tile
from concourse import bass_utils

F32 = mybir.dt.float32
BF16 = mybir.dt.bfloat16
I16 = mybir.dt.int16
AOP = mybir.AluOpType
AF = mybir.ActivationFunctionType

N_CORES = 8
B_FULL = 2048
C = 128
T = 256
L = 32
NJ = L + 1       # 33 gathered classes: [blank] + labels
S = 2 * L + 1    # 65 CTC states
NEX = 8          # examples per tile
THI = 16         # t-high bits on partitions
TLO = 16         # t-low bits in free dim
TPB = 16         # tiles per block
N_BLOCKS = 2     # blocks per core (256 examples)
B_CORE = N_BLOCKS * 128
PBIAS = -1.0     # p'' = exp(y + PBIAS); recenters alpha-max drift to ~0
LOSS_CONST = -T * PBIAS


def make_gidx(y_true_shard: np.ndarray) -> np.ndarray:
    """[ntiles, 128, NJ] int16 ap_gather index tensors (p-major wrapped per core)."""
    B = y_true_shard.shape[0]
    ntiles = B // NEX
    cls = np.concatenate(
        [np.zeros((B, 1), np.int64), y_true_shard.astype(np.int64)], axis=1
    )
    gidx = np.zeros((ntiles, 128, NJ), np.int16)
    i = np.arange(TLO * NJ)
    t_lo, j = i // NJ, i % NJ
    prow, pcol = i % 16, i // 16
    for k in range(ntiles):
        for e in range(NEX):
            gidx[k, 16 * e + prow, pcol] = t_lo * C + cls[k * NEX + e, j]
    return gidx


def make_mask(y_true_shard: np.ndarray) -> np.ndarray:
    """[nblocks, 128, L] f32 skip masks: mask[b, i] = (lab_i != lab_{i-1})."""
    B = y_true_shard.shape[0]
    m = np.zeros((B, L), np.float32)
    m[:, 1:] = (y_true_shard[:, 1:] != y_true_shard[:, :-1]).astype(np.float32)
    return m.reshape(B // 128, 128, L)


def scan3d(nc, out, data0, data1, initial, op0, op1):
    """tensor_tensor_scan allowing a 3-D strided data1 AP (verified on HW:
    the recurrence chains across AP dims in enumeration order)."""
    eng = nc.vector
    return eng.add_instruction(
        mybir.InstTensorScalarPtr(
            name=nc.get_next_instruction_name(),
            is_tensor_tensor_scan=True,
            is_scalar_tensor_tensor=True,
            op0=op0,
            op1=op1,
            ins=[
                eng.lower_ap(data0),
                eng.lower_ap_or_imm(initial),
                eng.lower_ap(data1),
            ],
            outs=[eng.lower_ap(out)],
        )
    )


def build_ctc(nc, loss_out, y_in, gidx_in, mask_in, n_blocks=N_BLOCKS):
    ntiles = n_blocks * TPB
    with tile.TileContext(nc) as tc:
        with (
            tc.tile_pool(name="io", bufs=3) as io_pool,
            tc.tile_pool(name="ztmp", bufs=2) as z_pool,
            tc.tile_pool(name="gat", bufs=2) as g_pool,
            tc.tile_pool(name="qraw", bufs=2) as q_pool,
            tc.tile_pool(name="persist", bufs=1) as pp,
            tc.tile_pool(name="ubuf", bufs=2) as u_pool,
            tc.tile_pool(name="dram", bufs=1, space="DRAM") as d_pool,
        ):
            logzacc = pp.tile([128, ntiles], F32, tag="logzacc")
            zall = pp.tile([128, ntiles * TLO], F32, tag="zall")
            scratch = d_pool.tile([ntiles * 128], F32, tag="scratch")

            p_pps = [
                pp.tile([128, T * NJ], BF16, tag=f"ppp{nb}", name=f"ppp{nb}")
                for nb in range(n_blocks)
            ]
            dpA = pp.tile([128, T + 1], BF16, tag="dpA")
            dpB = pp.tile([128, T + 1], BF16, tag="dpB")
            cde = [
                pp.tile([128, T + 1], BF16, tag=f"cde{i}", name=f"cde{i}")
                for i in range(3)
            ]
            onehot = pp.tile([128, T], BF16, tag="onehot")
            masks = [
                pp.tile([128, L], F32, tag=f"mask{nb}", name=f"mask{nb}")
                for nb in range(n_blocks)
            ]
            lzsum = [
                pp.tile([128, 1], F32, tag=f"lzsum{nb}", name=f"lzsum{nb}")
                for nb in range(n_blocks)
            ]
            biasln = pp.tile([128, 1], F32, tag="biasln")

            nc.vector.memset(biasln[:], PBIAS)
            nc.vector.memset(onehot[:], 0.0)
            nc.vector.memset(onehot[:, 0:1], 1.0)
            for b in (dpA, dpB, *cde):
                nc.vector.memset(b[:, 0:1], 0.0)
            for nb in range(n_blocks):
                nc.sync.dma_start(masks[nb][:], mask_in[nb])

            y4 = y_in.rearrange("(k e) t c -> k e t c", e=NEX)

            # ---- streaming phase ----
            qraws = []
            for nb in range(n_blocks):
                q_raw = q_pool.tile([128, T * NJ], F32, tag="qraw", name="qraw")
                qraws.append(q_raw)
                for kl in range(TPB):
                    k = nb * TPB + kl
                    ty = io_pool.tile([128, TLO * C], F32, tag="y")
                    src = y4[k].rearrange("e (th tl) c -> (e th) (tl c)", th=THI)
                    nc.sync.dma_start(ty[:], src)

                    te = z_pool.tile([128, TLO * C], BF16, tag="E")
                    nc.scalar.activation(te[:], ty[:], AF.Exp)
                    e3 = te[:].rearrange("p (tl c) -> p tl c", tl=TLO)
                    th = z_pool.tile([128, TLO * 64], BF16, tag="H")
                    h3 = th[:].rearrange("p (tl c) -> p tl c", tl=TLO)
                    nc.vector.tensor_tensor(
                        out=h3, in0=e3[:, :, 0:64], in1=e3[:, :, 64:128], op=AOP.add
                    )
                    nc.vector.tensor_reduce(
                        out=zall[:, k * TLO : (k + 1) * TLO], in_=h3,
                        axis=mybir.AxisListType.X, op=AOP.add,
                    )

                    tidx = g_pool.tile([128, NJ], I16, tag="idx")
                    nc.sync.dma_start(tidx[:], gidx_in[k])
                    tgat = g_pool.tile([128, TLO * NJ], F32, tag="gat")
                    nc.gpsimd.ap_gather(
                        out_ap=tgat[:],
                        in_ap=ty[:],
                        idxs_ap=tidx[:],
                        channels=128,
                        num_elems=TLO * C,
                        d=1,
                        num_idxs=TLO * NJ,
                    )
                    # fold: (e,th) partitions x (tl,j) -> [8 ex, th*528+tl*33+j]
                    dst = q_raw[:].rearrange("p (th w) -> p th w", th=THI)
                    nc.scalar.dma_start(dst[kl * NEX : (kl + 1) * NEX], tgat[:])

                nc.scalar.activation(
                    p_pps[nb][:], qraws[nb][:], AF.Exp, bias=biasln[:, 0:1]
                )

            # ---- logZ: one Ln pass, per-tile sums, partition fold via DRAM ----
            lnz = pp.tile([128, ntiles * TLO], F32, tag="lnz")
            nc.scalar.activation(lnz[:], zall[:], AF.Ln)
            lnz3 = lnz[:].rearrange("p (k tl) -> p k tl", tl=TLO)
            nc.vector.tensor_reduce(
                out=logzacc[:], in_=lnz3, axis=mybir.AxisListType.X, op=AOP.add
            )
            sc3 = scratch[:].rearrange("(k e th) -> e th k", e=NEX, th=THI)
            nc.sync.dma_start(sc3, logzacc[:])
            for nb in range(n_blocks):
                tlzs = z_pool.tile([128, THI], F32, tag="lzread")
                rb = scratch[:].rearrange(
                    "(nb kl e th) -> nb (kl e) th", nb=n_blocks, kl=TPB, e=NEX
                )
                nc.sync.dma_start(tlzs[:], rb[nb])
                nc.vector.tensor_reduce(
                    out=lzsum[nb][:], in_=tlzs[:], axis=mybir.AxisListType.X, op=AOP.add
                )

            # ---- DP phase ----
            for nb in range(n_blocks):
                p3 = p_pps[nb][:].rearrange(
                    "p (th tl j) -> p th tl j", th=THI, tl=TLO
                )

                def pcol(s):
                    j = 0 if s % 2 == 0 else 1 + (s - 1) // 2
                    return p3[:, :, :, j]

                scan3d(
                    nc, dpA[:, 1 : T + 1], onehot[:], pcol(0), 0.0,
                    AOP.add, AOP.mult,
                )
                tu1 = u_pool.tile([128, T], BF16, tag="U", name="tu1")
                nc.vector.scalar_tensor_tensor(
                    out=tu1[:], in0=onehot[:], scalar=1.0, in1=dpA[:, 0:T],
                    op0=AOP.mult, op1=AOP.add,
                )
                scan3d(
                    nc, dpB[:, 1 : T + 1], tu1[:], pcol(1), 0.0,
                    AOP.add, AOP.mult,
                )
                prev2, prev1 = dpA, dpB
                for s in range(2, S):
                    cur = cde[(s - 2) % 3]
                    if s % 2 == 0:
                        d0 = prev1[:, 0:T]
                    else:
                        i = (s - 1) // 2
                        tu = u_pool.tile([128, T], BF16, tag="U")
                        nc.vector.scalar_tensor_tensor(
                            out=tu[:], in0=prev2[:, 0:T],
                            scalar=masks[nb][:, i : i + 1],
                            in1=prev1[:, 0:T], op0=AOP.mult, op1=AOP.add,
                        )
                        d0 = tu[:]
                    scan3d(
                        nc, cur[:, 1 : T + 1], d0, pcol(s), 0.0,
                        AOP.add, AOP.mult,
                    )
                    prev2, prev1 = prev1, cur

                fin = z_pool.tile([128, 1], F32, tag="fin")
                nc.vector.tensor_tensor(
                    out=fin[:], in0=prev1[:, T : T + 1], in1=prev2[:, T : T + 1],
                    op=AOP.add,
                )
                lfin = z_pool.tile([128, 1], F32, tag="lfin")
                nc.scalar.activation(lfin[:], fin[:], AF.Ln)
                tloss = z_pool.tile([128, 1], F32, tag="loss")
                nc.vector.tensor_tensor(
                    out=tloss[:], in0=lzsum[nb][:], in1=lfin[:], op=AOP.subtract
                )
                nc.vector.tensor_scalar_add(
                    out=tloss[:], in0=tloss[:], scalar1=-LOSS_CONST
                )
                nc.sync.dma_start(loss_out[nb * 128 : (nb + 1) * 128], tloss[:, 0:1])


def _build_program():
    nc = bacc.Bacc("TRN2", num_devices=N_CORES, enable_partition_id=False)
    y_in = nc.dram_tensor("y", [B_CORE, T, C], F32, kind="ExternalInput").ap()
    gidx_in = nc.dram_tensor(
        "gidx", [N_BLOCKS * TPB, 128, NJ], I16, kind="ExternalInput"
    ).ap()
    mask_in = nc.dram_tensor(
        "mask", [N_BLOCKS, 128, L], F32, kind="ExternalInput"
    ).ap()
    loss_out = nc.dram_tensor("loss", [B_CORE], F32, kind="ExternalOutput").ap()
    build_ctc(nc, loss_out, y_in, gidx_in, mask_in)
    nc.compile()
    return nc


def kernel(y_true: np.ndarray, y_pred: np.ndarray, _trace: bool = False):
    y_true = np.asarray(y_true)
    y_pred = np.ascontiguousarray(np.asarray(y_pred, dtype=np.float32))
    assert y_pred.shape == (B_FULL, T, C) and y_true.shape == (B_FULL, L)

    nc = _build_program()
    in_maps = []
    for core in range(N_CORES):
        sl = slice(core * B_CORE, (core + 1) * B_CORE)
        yt = y_true[sl]
        in_maps.append(
            {
                "y": y_pred[sl],
                "gidx": make_gidx(yt),
                "mask": make_mask(yt),
            }
        )
    res = bass_utils.run_bass_kernel_spmd(
        nc, in_maps, core_ids=list(range(N_CORES)), trace=_trace
    )
    loss = np.concatenate([r["loss"] for r in res.results])
    if _trace:
        kernel._last_results = res
    return loss.astype(np.float32)


if __name__ == "__main__":
    rng = np.random.default_rng(0)
    yp = rng.standard_normal((B_FULL, T, C)).astype(np.float32)
    yt = rng.integers(1, C, (B_FULL, L)).astype(np.int32)
    out = kernel(yt, yp)
    print(out.shape, out[:4])
